# revision 39
# baseline (speedup 1.0000x reference)
"""Multi-head attention kernel for Trainium2 (8 NeuronCores, data-parallel over batch).

v5: 3-term compensated fp8 DoubleRow projections (see below) + restructured
schedule that eliminates the DMA-serialized head and the long tail:

 - One merged DMA per input kind (e.g. all 4 d-chunk-pair tiles of xqh in a
   single 3D-AP transfer) so Q/K chunk-0 projection starts ~6us in and the
   first exp fires ~11us in (was 45us: 32 serial HWDGE transfers).
 - V_ext phase runs inside the attention-group body (4 of 16 psum groups per
   group window) instead of as a serial prologue.
 - AV lags scores by ONE head pair, so after the last exp only AV(pair 7)
   and the output projection remain.

Numerics: Q/K/V projections are error-compensated 3-term fp8e4 DoubleRow
(out = xh@wh + xl@wh + xh@wl, host-split hi/lo, weights pre-scaled by 32 so
their residuals stay out of e4m3's subnormal range). DoubleRow = 256-deep
contraction at 0.5 cycles/row -> 49152 PE cycles per projection vs 65536
bf16. Scores/AV/out-proj stay bf16: fp8 there costs 2-3.5% max-rel error
(measured) vs the 2e-2 gate, and 3-term compensation is not cheaper than
bf16 on a 64-deep contraction.

Scale bookkeeping: qt/kt hold 32*(Q|K) in bf16 -> exp scale = SCALE/1024;
vext holds 32*V with its ones column memset to 32.0, so the softmax-rowsum
divide cancels the scale exactly.
"""

from contextlib import ExitStack

import numpy as np

import concourse.bass as bass
import concourse.mybir as mybir
import concourse.tile as tile
from concourse import bacc
from concourse.bass_utils import run_bass_kernel_spmd

F32 = mybir.dt.float32
BF = mybir.dt.bfloat16
FP8 = mybir.dt.float8e4
U16 = mybir.dt.uint16
DR = mybir.MatmulPerfMode.DoubleRow
ALU = mybir.AluOpType
ACTF = mybir.ActivationFunctionType

B, T, D, H = 8, 1024, 1024, 16
HD = D // H
SCALE = HD**-0.5
WS = 32.0  # host-side weight pre-scale (power of 2)
ESCALE = SCALE / (WS * WS)
OSCALE = 64.0  # evict-time OT scale (keeps fp8 residuals normal)
YSCALE = 1.0 / (WS * OSCALE)
P = 128
PT = D // P  # 8 chunks
HE = HD + 1  # 65
DE = H * HE  # 1040


def _build(esc_bufs=24, av_bufs=2, sc_bufs=2, pj_bufs=3, qt_bufs=3, ysb_bufs=2):
    nc = bacc.Bacc(None, target_bir_lowering=False)
    dr_ins = {}
    for nm in ("xqh", "xql", "xkh", "xkl", "xvh", "xvl"):
        dr_ins[nm] = nc.dram_tensor(nm, [4, P, 2 * T], FP8, kind="ExternalInput")
    for nm in ("wvh", "wvl"):
        dr_ins[nm] = nc.dram_tensor(nm, [4, P, 2 * D], FP8, kind="ExternalInput")
    for nm in ("wqh", "wql", "wkh", "wkl"):
        # column-block-major: [cb, p, (c2 kappa col)] so chunk-k weight slices
        # are 3D-contiguous DMAs
        dr_ins[nm] = nc.dram_tensor(nm, [PT, P, 8 * 128], FP8, kind="ExternalInput")
    woh_d = nc.dram_tensor("woh", [PT, P, D], FP8, kind="ExternalInput")
    wol_d = nc.dram_tensor("wol", [PT, P, D], FP8, kind="ExternalInput")
    bq_d = nc.dram_tensor("bq", [D], F32, kind="ExternalInput")  # 32*bq
    bk_d = nc.dram_tensor("bk", [D], F32, kind="ExternalInput")  # 32*bk
    bvh_d = nc.dram_tensor("bvh", [D], BF, kind="ExternalInput")  # 32*bv
    boh_d = nc.dram_tensor("boh", [D], BF, kind="ExternalInput")
    y_d = nc.dram_tensor("y", [T, D], F32, kind="ExternalOutput")

    with tile.TileContext(nc) as tc, ExitStack() as top:
        consts = top.enter_context(tc.tile_pool(name="consts", bufs=1, side="left"))
        bqT = consts.tile([P, PT], F32, tag="bqT")
        nc.gpsimd.dma_start(out=bqT, in_=bq_d[:].rearrange("(k p) -> p k", p=P))
        bkT = consts.tile([P, PT], F32, tag="bkT")
        nc.gpsimd.dma_start(out=bkT, in_=bk_d[:].rearrange("(k p) -> p k", p=P))
        bvb = consts.tile([P, D], BF, tag="bvb")
        nc.gpsimd.dma_start(
            out=bvb, in_=bass.AP(tensor=bvh_d, offset=0, ap=[[0, P], [1, D]])
        )
        bob = consts.tile([P, D], BF, tag="bob")
        nc.gpsimd.dma_start(
            out=bob, in_=bass.AP(tensor=boh_d, offset=0, ap=[[0, P], [1, D]])
        )

        ident = consts.tile([P, P], BF, tag="ident")
        from concourse.masks import make_identity

        make_identity(nc, ident)

        # persistent left pools
        vext_pool = top.enter_context(tc.tile_pool(name="vext", bufs=PT, side="left"))
        vext = [
            vext_pool.tile([P, DE], BF, tag="vext", name=f"vext{i}") for i in range(PT)
        ]
        for k in range(PT):
            # ones at WS/OSCALE: rowsum divide cancels the 32x V scale AND
            # applies the 64x OT scale for free
            nc.gpsimd.memset(
                vext[k].rearrange("p (h x) -> p h x", x=HE)[:, :, HD:HE], WS / OSCALE
            )
        otb_pool = top.enter_context(tc.tile_pool(name="otb", bufs=1, side="left"))
        otb = otb_pool.tile([P, PT * T * 2], FP8, tag="otb", name="otb")
        otbu3 = otb.bitcast(U16).rearrange("p (k t) -> p k t", t=T)

        # streaming pools (right side)
        qkp = top.enter_context(tc.tile_pool(name="qkp", bufs=8, side="right"))
        vwp = top.enter_context(tc.tile_pool(name="vwp", bufs=4, side="right"))
        qt_pool = top.enter_context(tc.tile_pool(name="qt", bufs=qt_bufs, side="right"))
        kt_pool = top.enter_context(tc.tile_pool(name="kt", bufs=qt_bufs, side="right"))
        esc_pool = top.enter_context(
            tc.tile_pool(name="esc", bufs=esc_bufs, side="right")
        )
        obq_pool = top.enter_context(tc.tile_pool(name="obq", bufs=2, side="right"))
        smalls = top.enter_context(tc.tile_pool(name="smalls", bufs=1, side="right"))
        ps = top.enter_context(tc.tile_pool(name="ps", bufs=1, space="PSUM"))

        # ---- input DMA: consumption-ordered chunks on one queue ----
        # Transfers serialize on the DMA engines, so order IS the schedule:
        # Q/K w-blocks 0-1 -> Q/K x streams -> V streams -> w rest -> wo.
        def kind_tile(pool, nm, tag):
            dram = dr_ins[nm]
            nf = dram.shape[2]
            tt = pool.tile([P, 4 * nf] if dram.shape[0] == 4 else [P, PT * nf],
                           FP8, tag=tag, name=nm)
            return tt, dram, nf

        def chunk_dma(tt, dram, nf, c2, n=1):
            nc.sync.dma_start(
                out=tt[:, c2 * nf : (c2 + n) * nf],
                in_=bass.AP(
                    tensor=dram,
                    offset=c2 * P * nf,
                    ap=[[nf, P], [P * nf, n], [1, nf]],
                ),
            )

        kinds = {}
        for nm in ("wqh", "wql", "wkh", "wkl", "xqh", "xql", "xkh", "xkl"):
            kinds[nm] = kind_tile(qkp, nm, "qk")
        for nm in ("xvh", "xvl", "wvh", "wvl"):
            kinds[nm] = kind_tile(vwp, nm, "vw")

        # 1. Q/K w column-blocks 0-1 (projection chunks 0 and 1)
        for nm in ("wqh", "wql", "wkh", "wkl"):
            tt, dram, nf = kinds[nm]
            chunk_dma(tt, dram, nf, 0, n=2)
        # 2. full Q/K x streams
        for c2 in range(4):
            for nm in ("xqh", "xkh"):
                tt, dram, nf = kinds[nm]
                chunk_dma(tt, dram, nf, c2)
        for c2 in range(4):
            for nm in ("xql", "xkl"):
                tt, dram, nf = kinds[nm]
                chunk_dma(tt, dram, nf, c2)
        # 3. V streams
        for c2 in range(4):
            for nm in ("xvh", "xvl"):
                tt, dram, nf = kinds[nm]
                chunk_dma(tt, dram, nf, c2)
        for c2 in range(4):
            for nm in ("wvh", "wvl"):
                tt, dram, nf = kinds[nm]
                chunk_dma(tt, dram, nf, c2)
        # 4. Q/K w column-blocks 2-3, then 4-7
        for nm in ("wqh", "wql", "wkh", "wkl"):
            tt, dram, nf = kinds[nm]
            chunk_dma(tt, dram, nf, 2, n=2)
        for nm in ("wqh", "wql", "wkh", "wkl"):
            tt, dram, nf = kinds[nm]
            chunk_dma(tt, dram, nf, 4, n=4)

        def wview(nm):
            # [p, cb, c2, kappa, col]
            tt = kinds[nm][0]
            return tt.rearrange(
                "p (cb c2 two col) -> p cb c2 two col", cb=PT, c2=4, two=2
            )

        def xview(nm):
            tt = kinds[nm][0]
            r = tt.rearrange("p (c two t) -> p c two t", c=4, two=2)
            return [r[:, c2] for c2 in range(4)]

        qk = {nm: wview(nm) for nm in ("wqh", "wql", "wkh", "wkl")}
        qk.update({nm: xview(nm) for nm in ("xqh", "xql", "xkh", "xkl")})
        xvh = xview("xvh")
        xvl = xview("xvl")
        wvh = xview("wvh")
        wvl = xview("wvl")

        # 5. wo hi/lo: two 8KB fp8 tiles rotating into the V slots
        wo5 = {}
        for nm, dram in (("woh", woh_d), ("wol", wol_d)):
            wt = vwp.tile([P, PT * D], FP8, tag="vw", name=nm)
            nc.sync.dma_start(
                out=wt,
                in_=bass.AP(
                    tensor=dram, offset=0, ap=[[D, P], [P * D, PT], [1, D]]
                ),
            )
            wo5[nm] = wt.rearrange("p (kp two t) -> p kp two t", kp=4, two=2)

        # ---- item factories ----

        def v_items():
            """16 items in ci-major order: all 8 s-blocks of i-half 0 first
            (heads 0-7), then i-half 1 — so AV pair p only needs the first
            half once p < 4."""
            items = []

            def mk(k, ci):
                def run():
                    pv = ps.tile([P, 512], F32, tag="pj", bufs=pj_bufs, name=f"pv{k}_{ci}")
                    for c2 in range(4):
                        for ti, (xs, ws) in enumerate(
                            ((xvh, wvh), (xvl, wvh), (xvh, wvl))
                        ):
                            nc.tensor.matmul(
                                pv[:, :],
                                xs[c2][:, :, 128 * k : 128 * (k + 1)],
                                ws[c2][:, :, 512 * ci : 512 * (ci + 1)],
                                start=(c2 == 0 and ti == 0),
                                stop=(c2 == 3 and ti == 2),
                                perf_mode=DR,
                            )
                    nc.vector.tensor_tensor(
                        out=vext[k].rearrange("p (h x) -> p h x", x=HE)[
                            :, 8 * ci : 8 * (ci + 1), 0:HD
                        ],
                        in0=pv.rearrange("p (h x) -> p h x", x=HD),
                        in1=bvb[:, 512 * ci : 512 * (ci + 1)].rearrange(
                            "p (h x) -> p h x", x=HD
                        ),
                        op=ALU.add,
                    )

                return run

            for ci in range(2):
                for k in range(PT):
                    items.append(mk(k, ci))
            return items

        qt = {}
        kt = {}

        def proj_items(dst, k, wh, wl, xh, xl, bias):
            """4 closures; each emits 6 of the 12 DoubleRow instrs of one
            512-col projection (c2-pairs 01 / 23)."""
            items = []
            state = {}

            def mk(c, half):
                def run():
                    if half == 0:
                        state[c] = ps.tile([P, 512], F32, tag="pj", bufs=pj_bufs, name=f"pj{k}_{c}")
                    pt_ = state[c]
                    for c2 in range(2 * half, 2 * half + 2):
                        for ti, (ws_, xs_) in enumerate(
                            ((wh, xh), (wh, xl), (wl, xh))
                        ):
                            nc.tensor.matmul(
                                pt_[:, :],
                                ws_[:, k, c2],
                                xs_[c2][:, :, 512 * c : 512 * (c + 1)],
                                start=(c2 == 0 and ti == 0),
                                stop=(c2 == 3 and ti == 2),
                                perf_mode=DR,
                            )
                    if half == 1:
                        nc.vector.tensor_scalar(
                            out=dst[:, 512 * c : 512 * (c + 1)],
                            in0=pt_[:, :],
                            scalar1=bias[:, k : k + 1],
                            scalar2=None,
                            op0=ALU.add,
                        )

                return run

            for c in range(2):
                items.append(mk(c, 0))
                items.append(mk(c, 1))
            return items

        def make_qk_items(k):
            qt[k] = qt_pool.tile([P, T], BF, tag="qt", name=f"qt{k}")
            kt[k] = kt_pool.tile([P, T], BF, tag="kt", name=f"kt{k}")
            return proj_items(
                qt[k], k, qk["wqh"], qk["wql"], qk["xqh"], qk["xql"], bqT
            ) + proj_items(
                kt[k], k, qk["wkh"], qk["wkl"], qk["xkh"], qk["xkl"], bkT
            )

        esc = {}

        def make_sc_items(h):
            hi, ro = h // 2, 64 * (h % 2)
            esc[h] = []

            def mk(s):
                def run():
                    psc = ps.tile([P, T], F32, tag="sc", bufs=sc_bufs, name=f"sc{h}_{s}")
                    for c in range(2):
                        nc.tensor.matmul(
                            psc[:, 512 * c : 512 * (c + 1)],
                            kt[hi][ro : ro + 64, 128 * s : 128 * (s + 1)],
                            qt[hi][ro : ro + 64, 512 * c : 512 * (c + 1)],
                            start=True,
                            stop=True,
                        )
                    e = esc_pool.tile([P, T], BF, tag="esc", name=f"esc{h}_{s}")
                    nc.scalar.activation(out=e, in_=psc[:, :], func=ACTF.Exp, scale=ESCALE)
                    esc[h].append(e)

                return run

            return [mk(s) for s in range(PT)]

        obq = {}
        pav_dbuf = ps.tile([P, 2 * HE], F32, tag="av", bufs=1, name="pav_dbuf")
        av_ctr = [0]

        def make_av_items(h):
            q = h // 4
            if q not in obq:
                # [tm, 4 heads x 64 i x (hi,lo) bytes] fp8, u16-packed pairs
                t_ = obq_pool.tile([P, PT * 512], FP8, tag="ob", bufs=2, name=f"ob{q}")
                obq[q] = t_
            ob = obq[q]
            obu3 = ob.bitcast(U16).rearrange("p (t i) -> p t i", i=256)
            bcol = 128 * (h % 4)  # byte offset of this head in a tm block

            def mk(tm):
                def run():
                    par = av_ctr[0] % 2
                    av_ctr[0] += 1
                    pav = pav_dbuf[:, HE * par : HE * (par + 1)]
                    for s in range(PT):
                        nc.tensor.matmul(
                            pav[:, :],
                            esc[h][s][:, 128 * tm : 128 * (tm + 1)],
                            vext[s][:, HE * h : HE * (h + 1)],
                            start=(s == 0),
                            stop=(s == PT - 1),
                            skip_group_check=True,
                        )
                    # free pav fast: copy to SBUF, then quantize on gpsimd
                    tmp = smalls.tile([P, HE], F32, tag="avt", bufs=4, name=f"avt{h}_{tm}")
                    nc.vector.tensor_copy(tmp, pav)
                    rcp = smalls.tile([P, 1], F32, tag="rcp", bufs=6, name=f"rcp{h}_{tm}")
                    nc.vector.reciprocal(rcp, tmp[:, HD : HD + 1])
                    obh = bass.AP(
                        tensor=ob.tensor,
                        offset=ob.offset + 512 * tm + bcol,
                        ap=[ob.ap[0], [2, HD]],
                    )
                    obl = bass.AP(
                        tensor=ob.tensor,
                        offset=ob.offset + 512 * tm + bcol + 1,
                        ap=[ob.ap[0], [2, HD]],
                    )
                    nc.vector.tensor_scalar(
                        out=obh,
                        in0=tmp[:, 0:HD],
                        scalar1=rcp,
                        scalar2=None,
                        op0=ALU.mult,
                    )
                    nc.vector.scalar_tensor_tensor(
                        out=obl,
                        in0=tmp[:, 0:HD],
                        scalar=rcp,
                        in1=obh,
                        op0=ALU.mult,
                        op1=ALU.subtract,
                    )
                    if h % 2 == 1:
                        p_ = h // 2
                        nc.sync.dma_start_transpose(
                            out=otbu3[:, p_, 128 * tm : 128 * (tm + 1)],
                            in_=obu3[:, tm, 128 * (p_ % 2) : 128 * (p_ % 2) + 128],
                        )

                return run

            return [mk(tm) for tm in range(PT)]

        # ---- output projection helpers (2-wave) ----
        # wave A (k<4) runs inside group 7's ACT window, accumulating into
        # SBUF tiles that reuse freed qk-input slots; wave B (k>=4) runs at
        # the tail interleaved with AV pair 7.
        yacc = {}

        def emit_oproj(psy, c, m, kp0, kp1):
            """3-term fp8 DoubleRow rank-update: k-pairs [kp0, kp1)."""
            n = 0
            nt = (kp1 - kp0) * 3
            for kp in range(kp0, kp1):
                for osel, wnm in ((0, "woh"), (1, "woh"), (0, "wol")):
                    lhs = bass.AP(
                        tensor=otb.tensor,
                        offset=otb.offset + kp * 4 * T + 2 * (128 * m) + osel,
                        ap=[otb.ap[0], [2 * T, 2], [2, 128]],
                    )
                    nc.tensor.matmul(
                        psy,
                        lhs,
                        wo5[wnm][:, kp, :, 512 * c : 512 * (c + 1)],
                        start=(n == 0),
                        stop=(n == nt - 1),
                        perf_mode=DR,
                    )
                    n += 1

        def wave_items(kp0, kp1, first):
            items = []

            def mk(c, m):
                def run():
                    j = (2 * m + c) // 4  # 4 acc tiles of 4 chunks each
                    if j not in yacc:
                        t_ = qkp.tile([P, 2 * T], F32, tag="qk", name=f"yacc{j}")
                        yacc[j] = t_.rearrange("p (s f) -> p s f", f=512)
                    psy = ps.tile([P, 512], F32, tag="pj", bufs=pj_bufs, name=f"py{kp0}_{c}_{m}")
                    emit_oproj(psy, c, m, kp0, kp1)
                    sl = yacc[j][:, (2 * m + c) % 4, :]
                    nc.vector.scalar_tensor_tensor(
                        out=sl,
                        in0=psy,
                        scalar=YSCALE,
                        in1=bob[:, 512 * c : 512 * (c + 1)] if first else sl,
                        op0=ALU.mult,
                        op1=ALU.add,
                    )

                return run

            for m in range(PT):
                for c in range(2):
                    items.append(mk(c, m))
            return items

        def waveb_item(c, m):
            def run():
                if m % 2 == 0:
                    psy = ps.tile([P, 512], F32, tag="pj", bufs=pj_bufs, name=f"pyb{c}_{m}")
                else:
                    pyt = ps.tile([P, T], F32, tag="sc", bufs=sc_bufs, name=f"pyb{c}_{m}")
                    psy = pyt[:, 0:512]
                emit_oproj(psy, c, m, 3, 4)
                j = (2 * m + c) // 4
                ysb = smalls.tile([P, 512], F32, tag="ysb", bufs=ysb_bufs, name=f"ysb{c}_{m}")
                nc.vector.scalar_tensor_tensor(
                    out=ysb,
                    in0=psy,
                    scalar=YSCALE,
                    in1=yacc[j][:, (2 * m + c) % 4, :],
                    op0=ALU.mult,
                    op1=ALU.add,
                )
                nc.scalar.dma_start(
                    out=y_d[128 * m : 128 * (m + 1), 512 * c : 512 * (c + 1)],
                    in_=ysb,
                )

            return run

        # ---- schedule ----
        # pre-loop: QT(0)/KT(0) projections only (enabled ~6us in by the
        # merged Q-kind DMAs).
        vit = v_items()
        for it in make_qk_items(0):
            it()

        # groups: scores+exp for pair k; V_ext groups fill groups 0-1 (all
        # emitted before any AV reads vext); AV lags by TWO pairs from group
        # 2; projection for pair k+1; wave-A out-proj inside group 7.
        av_sched = {k: (k - 1,) for k in range(1, PT)}
        for k in range(PT):
            sc_items = make_sc_items(2 * k) + make_sc_items(2 * k + 1)
            qk_items = make_qk_items(k + 1) if k < PT - 1 else []
            av_items = []
            for pr in av_sched.get(k, ()):
                av_items += make_av_items(2 * pr) + make_av_items(2 * pr + 1)
            vslice = vit[8 * k : 8 * (k + 1)] if k <= 1 else []
            for i in range(16):
                sc_items[i]()
                if vslice and i % 2 == 0:
                    vslice[i // 2]()
                for av in av_items[i::16]:
                    av()
                if qk_items and i % 2 == 1:
                    qk_items[(i - 1) // 2]()
            # out-proj waves after the group's projection reads (yacc tiles
            # reuse the w-kind input slots)
            if k == PT - 2:
                for it in wave_items(0, 2, True):
                    it()
            elif k == PT - 1:
                for it in wave_items(2, 3, False):
                    it()

        # tail: AV pairs 6 and 7; pair 7's transposes run on the PE and its
        # per-tm completion releases the matching wave-B out-proj chunk.
        av7 = make_av_items(14) + make_av_items(15)
        for tm in range(PT):
            av7[tm]()
            av7[8 + tm]()
            waveb_item(0, tm)()
            waveb_item(1, tm)()

    nc.compile()
    return nc


_NC_CACHE = None


def _get_nc():
    global _NC_CACHE
    if _NC_CACHE is None:
        _NC_CACHE = _build()
    return _NC_CACHE


def _pairs(a):
    """[1024, n] -> [4, 128, 2n]: d-chunk pairs, k-halves along free dim."""
    n = a.shape[1]
    return np.ascontiguousarray(
        a.reshape(4, 2, 128, n).transpose(0, 2, 1, 3).reshape(4, 128, 2 * n)
    )


def kernel(**inputs) -> np.ndarray:
    import ml_dtypes

    bf16 = ml_dtypes.bfloat16
    e4m3 = ml_dtypes.float8_e4m3

    def split_pairs(a):
        hi = a.astype(e4m3)
        lo = (a - hi.astype(np.float32)).astype(e4m3)
        return _pairs(hi), _pairs(lo)

    query = np.asarray(inputs["query"], dtype=np.float32)
    key = np.asarray(inputs["key"], dtype=np.float32)
    value = np.asarray(inputs["value"], dtype=np.float32)

    def split_wcb(a):  # [d, i] -> hi, lo in [cb, p, (c2 kappa col)] layout
        hi = a.astype(e4m3)
        lo = (a - hi.astype(np.float32)).astype(e4m3)

        def pack(w):
            # [256*c2+128*kappa+p, 128*cb+col] -> [cb, p, c2, kappa, col]
            w5 = w.reshape(4, 2, 128, PT, 128).transpose(3, 2, 0, 1, 4)
            return np.ascontiguousarray(w5.reshape(PT, 128, 8 * 128))

        return pack(hi), pack(lo)

    wqh, wql = split_wcb(np.asarray(inputs["Wq"], np.float32).T * WS)
    wkh, wkl = split_wcb(np.asarray(inputs["Wk"], np.float32).T * WS)
    wvh, wvl = split_pairs(np.asarray(inputs["Wv"], np.float32).T * WS)
    woT32 = np.ascontiguousarray(np.asarray(inputs["Wo"], np.float32).T) * WS
    woh_f = woT32.astype(e4m3)
    wol_f = (woT32 - woh_f.astype(np.float32)).astype(e4m3)
    woh = np.ascontiguousarray(woh_f.reshape(PT, 128, D))
    wol = np.ascontiguousarray(wol_f.reshape(PT, 128, D))

    bq = np.ascontiguousarray(np.asarray(inputs["bq"], np.float32) * WS)
    bk = np.ascontiguousarray(np.asarray(inputs["bk"], np.float32) * WS)
    bvh = (np.asarray(inputs["bv"], np.float32) * WS).astype(bf16)
    boh = np.asarray(inputs["bo"], np.float32).astype(bf16)

    nc = _get_nc()
    in_maps = []
    for b in range(B):
        xqh, xql = split_pairs(np.ascontiguousarray(query[b].T))
        xkh, xkl = split_pairs(np.ascontiguousarray(key[b].T))
        xvh, xvl = split_pairs(np.ascontiguousarray(value[b].T))
        in_maps.append(
            {
                "xqh": xqh, "xql": xql,
                "xkh": xkh, "xkl": xkl,
                "xvh": xvh, "xvl": xvl,
                "wqh": wqh, "wql": wql,
                "wkh": wkh, "wkl": wkl,
                "wvh": wvh, "wvl": wvl,
                "woh": woh, "wol": wol,
                "bq": bq, "bk": bk, "bvh": bvh, "boh": boh,
            }
        )
    res = run_bass_kernel_spmd(nc, in_maps, core_ids=list(range(B)))
    return np.stack([res.results[b]["y"] for b in range(B)], axis=0)
